# revision 5
# baseline (speedup 1.0000x reference)
"""Trainium2 Bass kernel for LuongAttnDecoderRNN (single GRU step + dot
attention + two 32k-vocab projections), SPMD over 8 NeuronCores.

Sharding:
  - Kernel 1: data-parallel over batch B (8 rows per core) for GRU,
    attention scores, softmax, context and the concat projection.
  - Kernel 2: tensor-parallel over the vocab dim for the two [32000, 1024]
    output projections (4000 pinyin + 4000 char columns per core, padded
    to 4096).
Host does only data marshalling: embedding row gather, transposes/casts to
fp16, padding, shard assembly.  All FLOPs run on device; fp32 PSUM
accumulation everywhere (operands fp16).
"""

import os
import sys
import time
from contextlib import ExitStack

import numpy as np

sys.path.insert(0, "/opt/trn_rl_repo")

import concourse.bass as bass  # noqa: E402
import concourse.bacc as bacc  # noqa: E402
import concourse.mybir as mybir  # noqa: E402
import concourse.tile as tile  # noqa: E402

dt = mybir.dt
AFT = mybir.ActivationFunctionType

N_CORES = 8
B, H, L, V = 64, 1024, 1024, 32000
BL = B // N_CORES          # 8 batch rows per core
VL = V // N_CORES          # 4000 vocab cols per head per core
VLP = 4096                 # padded
F16 = np.float16
F32 = np.float32

# timing info for the last kernel() call, readable by test.py
LAST_EXEC_NS = {}
LAST_PROFILE = {}

_BUILD_CACHE = {}


def _new_nc():
    return bacc.Bacc(
        "TRN2",
        target_bir_lowering=False,
        debug=False,
        enable_asserts=False,
        num_devices=N_CORES,
    )


# --------------------------------------------------------------------------
# Kernel 1: GRU + attention + concat, B-sharded (8 rows per core)
# --------------------------------------------------------------------------
def _build_k1():
    nc = _new_nc()
    f16, f32 = dt.float16, dt.float32

    xht = nc.dram_tensor("xht", [2 * H, BL], f16, kind="ExternalInput").ap()
    h0 = nc.dram_tensor("h0", [BL, H], f32, kind="ExternalInput").ap()
    wrz = nc.dram_tensor("wrz", [2 * H, 2 * H], f16, kind="ExternalInput").ap()
    wni = nc.dram_tensor("wni", [H, H], f16, kind="ExternalInput").ap()
    wnh = nc.dram_tensor("wnh", [H, H], f16, kind="ExternalInput").ap()
    brz = nc.dram_tensor("brz", [BL, 2 * H], f32, kind="ExternalInput").ap()
    bin_ = nc.dram_tensor("bin", [BL, H], f32, kind="ExternalInput").ap()
    bhn = nc.dram_tensor("bhn", [BL, H], f32, kind="ExternalInput").ap()
    et = nc.dram_tensor("et", [8, 128, BL, L], f16, kind="ExternalInput").ap()
    en = nc.dram_tensor("en", [8, 128, BL, H], f16, kind="ExternalInput").ap()
    cwt = nc.dram_tensor("cwt", [2 * H, H], f16, kind="ExternalInput").ap()
    cb = nc.dram_tensor("cb", [BL, H], f32, kind="ExternalInput").ap()
    ident = nc.dram_tensor("ident", [128, 128], f16, kind="ExternalInput").ap()

    hid_o = nc.dram_tensor("hid_o", [BL, H], f32, kind="ExternalOutput").ap()
    attn_o = nc.dram_tensor("attn_o", [BL, L], f32, kind="ExternalOutput").ap()
    co_o = nc.dram_tensor("co_o", [BL, H], f32, kind="ExternalOutput").ap()

    KT = 2 * H // 128  # 16 k-tiles over the [x;h] stacked contraction

    with tile.TileContext(nc) as tc, ExitStack() as ctx:
        const = ctx.enter_context(tc.tile_pool(name="const", bufs=1))
        wpool = ctx.enter_context(tc.tile_pool(name="w", bufs=6))
        epool = ctx.enter_context(tc.tile_pool(name="e", bufs=6))
        sm = ctx.enter_context(tc.tile_pool(name="sm", bufs=2))
        keep = ctx.enter_context(tc.tile_pool(name="keep", bufs=1))
        psA = ctx.enter_context(tc.tile_pool(name="psA", bufs=1, space="PSUM"))
        psB = ctx.enter_context(tc.tile_pool(name="psB", bufs=2, space="PSUM"))

        # ---- constants in ----
        id_sb = const.tile([128, 128], f16, tag="ident")
        nc.sync.dma_start(id_sb[:, :], ident)
        xh_sb = const.tile([128, KT, BL], f16, tag="xht")
        nc.sync.dma_start(xh_sb[:, :, :], xht.rearrange("(kt p) b -> p kt b", p=128))
        h_sb = const.tile([BL, H], f32, tag="h0")
        nc.sync.dma_start(h_sb[:, :], h0)
        brz_sb = const.tile([BL, 2 * H], f32, tag="brz")
        nc.sync.dma_start(brz_sb[:, :], brz)
        bin_sb = const.tile([BL, H], f32, tag="bin")
        nc.sync.dma_start(bin_sb[:, :], bin_)
        bhn_sb = const.tile([BL, H], f32, tag="bhn")
        nc.sync.dma_start(bhn_sb[:, :], bhn)
        cb_sb = const.tile([BL, H], f32, tag="cb")
        nc.sync.dma_start(cb_sb[:, :], cb)

        # ---- GRU matmuls: rz (stacked 2048-contraction), gi_n, gh_n ----
        rz_ps = psA.tile([BL, 2 * H], f32, tag="A")
        for nch in range(4):
            for kt in range(KT):
                w_t = wpool.tile([128, 512], f16, tag="w")
                nc.sync.dma_start(
                    w_t[:, :], wrz[kt * 128:(kt + 1) * 128, nch * 512:(nch + 1) * 512]
                )
                nc.tensor.matmul(
                    rz_ps[:, nch * 512:(nch + 1) * 512],
                    xh_sb[:, kt, :], w_t[:, :],
                    start=(kt == 0), stop=(kt == KT - 1),
                )
        gin_ps = psB.tile([BL, H], f32, tag="B")
        ghn_ps = psB.tile([BL, H], f32, tag="B")
        for nch in range(2):
            for kt in range(8):
                w_t = wpool.tile([128, 512], f16, tag="w")
                nc.sync.dma_start(
                    w_t[:, :], wni[kt * 128:(kt + 1) * 128, nch * 512:(nch + 1) * 512]
                )
                nc.tensor.matmul(
                    gin_ps[:, nch * 512:(nch + 1) * 512],
                    xh_sb[:, kt, :], w_t[:, :],
                    start=(kt == 0), stop=(kt == 7),
                )
        for nch in range(2):
            for kt in range(8):
                w_t = wpool.tile([128, 512], f16, tag="w")
                nc.sync.dma_start(
                    w_t[:, :], wnh[kt * 128:(kt + 1) * 128, nch * 512:(nch + 1) * 512]
                )
                nc.tensor.matmul(
                    ghn_ps[:, nch * 512:(nch + 1) * 512],
                    xh_sb[:, 8 + kt, :], w_t[:, :],
                    start=(kt == 0), stop=(kt == 7),
                )

        # ---- GRU elementwise ----
        sig = sm.tile([BL, 2 * H], f32, tag="sig")
        nc.vector.tensor_add(sig[:, :], rz_ps[:, :], brz_sb[:, :])
        rs = sm.tile([BL, 2 * H], f32, tag="rs")
        nc.scalar.activation(rs[:, :], sig[:, :], AFT.Sigmoid)
        t_in = sm.tile([BL, H], f32, tag="tin")
        nc.vector.tensor_add(t_in[:, :], gin_ps[:, :], bin_sb[:, :])
        t_hn = sm.tile([BL, H], f32, tag="thn")
        nc.vector.tensor_add(t_hn[:, :], ghn_ps[:, :], bhn_sb[:, :])
        t_rn = sm.tile([BL, H], f32, tag="trn")
        nc.vector.tensor_mul(t_rn[:, :], rs[:, 0:H], t_hn[:, :])
        n_pre = sm.tile([BL, H], f32, tag="npre")
        nc.vector.tensor_add(n_pre[:, :], t_in[:, :], t_rn[:, :])
        n_t = sm.tile([BL, H], f32, tag="nt")
        nc.scalar.activation(n_t[:, :], n_pre[:, :], AFT.Tanh)
        d_t = sm.tile([BL, H], f32, tag="dt")
        nc.vector.tensor_sub(d_t[:, :], h_sb[:, :], n_t[:, :])
        zd = sm.tile([BL, H], f32, tag="zd")
        nc.vector.tensor_mul(zd[:, :], rs[:, H:2 * H], d_t[:, :])
        h_new = keep.tile([BL, H], f32, tag="hnew")
        nc.vector.tensor_add(h_new[:, :], n_t[:, :], zd[:, :])
        nc.sync.dma_start(hid_o, h_new[:, :])
        h_new16 = keep.tile([BL, H], f16, tag="hnew16")
        nc.vector.tensor_copy(h_new16[:, :], h_new[:, :])

        # ---- h_newT via PE transpose: [8, 1024] -> 8 chunks [128, 8] ----
        hT_sb = keep.tile([128, 8, BL], f16, tag="hT")
        for hc in range(8):
            tp = psB.tile([128, BL], f16, tag="B")
            nc.tensor.transpose(
                tp[:, :], h_new16[:, hc * 128:(hc + 1) * 128], id_sb[0:BL, 0:BL]
            )
            nc.vector.tensor_copy(hT_sb[:, hc, :], tp[:, :])

        # ---- scores: per-b [1, 1024] psum; pack to [8, 1024] via sbuf DMA --
        sc_pack = keep.tile([BL, L], f32, tag="scpack")
        for b in range(BL):
            sc_ps = psB.tile([1, L], f32, tag="B")
            for lc in range(2):
                for ht in range(8):
                    et_t = epool.tile([128, 512], f16, tag="et")
                    nc.sync.dma_start(
                        et_t[:, :], et[ht, :, b, lc * 512:(lc + 1) * 512]
                    )
                    nc.tensor.matmul(
                        sc_ps[:, lc * 512:(lc + 1) * 512],
                        hT_sb[:, ht, b:b + 1], et_t[:, :],
                        start=(ht == 0), stop=(ht == 7),
                    )
            sc_row = sm.tile([1, L], f32, tag="scrow")
            nc.scalar.copy(sc_row[:, :], sc_ps[:, :])
            nc.sync.dma_start(sc_pack[b:b + 1, :], sc_row[:, :])

        # ---- softmax over l (free axis) ----
        neg_mx = sm.tile([BL, 1], f32, tag="negmx")
        nc.vector.reduce_max(
            neg_mx[:, :], sc_pack[:, :], axis=mybir.AxisListType.X, negate=True
        )
        exp_s = sm.tile([BL, L], f32, tag="exps")
        sum_s = sm.tile([BL, 1], f32, tag="sums")
        nc.scalar.activation(
            exp_s[:, :], sc_pack[:, :], AFT.Exp,
            bias=neg_mx[:, :], scale=1.0, accum_out=sum_s[:, :],
        )
        rcp = sm.tile([BL, 1], f32, tag="rcp")
        nc.vector.reciprocal(rcp[:, :], sum_s[:, :])
        attn_f = keep.tile([BL, L], f32, tag="attnf")
        nc.vector.tensor_scalar_mul(attn_f[:, :], exp_s[:, :], rcp[:, :])
        nc.sync.dma_start(attn_o, attn_f[:, :])
        attn16 = keep.tile([BL, L], f16, tag="attn16")
        nc.vector.tensor_copy(attn16[:, :], attn_f[:, :])

        # ---- attnT via PE transpose: 8 chunks [128 l, 8 b] ----
        aT_sb = keep.tile([128, 8, BL], f16, tag="aT")
        for lt in range(8):
            tp = psB.tile([128, BL], f16, tag="B")
            nc.tensor.transpose(
                tp[:, :], attn16[:, lt * 128:(lt + 1) * 128], id_sb[0:BL, 0:BL]
            )
            nc.vector.tensor_copy(aT_sb[:, lt, :], tp[:, :])

        # ---- context: per-b [1, 1024] psum; pack to [8, 1024] ----
        ctx_pack = keep.tile([BL, H], f32, tag="ctxpack")
        for b in range(BL):
            ctx_ps = psB.tile([1, H], f32, tag="B")
            for lt in range(8):
                en_t = epool.tile([128, H], f16, tag="en")
                nc.sync.dma_start(en_t[:, :], en[lt, :, b, :])
                for hc in range(2):
                    nc.tensor.matmul(
                        ctx_ps[:, hc * 512:(hc + 1) * 512],
                        aT_sb[:, lt, b:b + 1], en_t[:, hc * 512:(hc + 1) * 512],
                        start=(lt == 0), stop=(lt == 7),
                    )
            ctx_row = sm.tile([1, H], f32, tag="ctxrow")
            nc.scalar.copy(ctx_row[:, :], ctx_ps[:, :])
            nc.sync.dma_start(ctx_pack[b:b + 1, :], ctx_row[:, :])

        ctx16 = keep.tile([BL, H], f16, tag="ctx16")
        nc.vector.tensor_copy(ctx16[:, :], ctx_pack[:, :])
        cT_sb = keep.tile([128, 8, BL], f16, tag="cT")
        for hc in range(8):
            tp = psB.tile([128, BL], f16, tag="B")
            nc.tensor.transpose(
                tp[:, :], ctx16[:, hc * 128:(hc + 1) * 128], id_sb[0:BL, 0:BL]
            )
            nc.vector.tensor_copy(cT_sb[:, hc, :], tp[:, :])

        # ---- concat projection: co = tanh([h_new, ctx] @ cw.T + cb) ----
        co_ps = psA.tile([BL, H], f32, tag="A")
        for nch in range(2):
            for kt in range(KT):
                w_t = wpool.tile([128, 512], f16, tag="w")
                nc.sync.dma_start(
                    w_t[:, :], cwt[kt * 128:(kt + 1) * 128, nch * 512:(nch + 1) * 512]
                )
                lhsT = hT_sb[:, kt, :] if kt < 8 else cT_sb[:, kt - 8, :]
                nc.tensor.matmul(
                    co_ps[:, nch * 512:(nch + 1) * 512],
                    lhsT, w_t[:, :],
                    start=(kt == 0), stop=(kt == KT - 1),
                )
        co_pre = sm.tile([BL, H], f32, tag="copre")
        nc.vector.tensor_add(co_pre[:, :], co_ps[:, :], cb_sb[:, :])
        co_f = sm.tile([BL, H], f32, tag="cof")
        nc.scalar.activation(co_f[:, :], co_pre[:, :], AFT.Tanh)
        nc.sync.dma_start(co_o, co_f[:, :])

    nc.compile()
    return nc


# --------------------------------------------------------------------------
# Kernel 2: vocab-sharded output projections
# --------------------------------------------------------------------------
def _build_k2():
    nc = _new_nc()
    f16, f32 = dt.float16, dt.float32

    cot = nc.dram_tensor("cot", [H, B], f16, kind="ExternalInput").ap()
    wpt = nc.dram_tensor("wpt", [H, VLP], f16, kind="ExternalInput").ap()
    wct = nc.dram_tensor("wct", [H, VLP], f16, kind="ExternalInput").ap()
    bp = nc.dram_tensor("bp", [B, VLP], f32, kind="ExternalInput").ap()
    bc = nc.dram_tensor("bc", [B, VLP], f32, kind="ExternalInput").ap()
    outp = nc.dram_tensor("outp", [B, VLP], f32, kind="ExternalOutput").ap()
    outc = nc.dram_tensor("outc", [B, VLP], f32, kind="ExternalOutput").ap()

    with tile.TileContext(nc) as tc, ExitStack() as ctx:
        const = ctx.enter_context(tc.tile_pool(name="const", bufs=1))
        wpool = ctx.enter_context(tc.tile_pool(name="w", bufs=8))
        opool = ctx.enter_context(tc.tile_pool(name="o", bufs=4))
        psp = ctx.enter_context(tc.tile_pool(name="ps", bufs=4, space="PSUM"))

        co_sb = const.tile([128, 8, B], f16, tag="cot")
        nc.sync.dma_start(co_sb[:, :, :], cot.rearrange("(kt p) b -> p kt b", p=128))
        bp_sb = const.tile([B, VLP], f32, tag="bp")
        nc.sync.dma_start(bp_sb[:, :], bp)
        bc_sb = const.tile([B, VLP], f32, tag="bc")
        nc.sync.dma_start(bc_sb[:, :], bc)

        for w_dram, b_sb, o_dram in ((wpt, bp_sb, outp), (wct, bc_sb, outc)):
            for nch in range(VLP // 512):
                o_ps = psp.tile([B, 512], f32, tag="ops")
                for kt in range(8):
                    w_t = wpool.tile([128, 512], f16, tag="w")
                    nc.sync.dma_start(
                        w_t[:, :],
                        w_dram[kt * 128:(kt + 1) * 128, nch * 512:(nch + 1) * 512],
                    )
                    nc.tensor.matmul(
                        o_ps[:, :], co_sb[:, kt, :], w_t[:, :],
                        start=(kt == 0), stop=(kt == 7),
                    )
                o_sb = opool.tile([B, 512], f32, tag="osb")
                nc.vector.tensor_add(
                    o_sb[:, :], o_ps[:, :], b_sb[:, nch * 512:(nch + 1) * 512]
                )
                nc.sync.dma_start(o_dram[:, nch * 512:(nch + 1) * 512], o_sb[:, :])

    nc.compile()
    return nc


def _get(name, builder):
    if name not in _BUILD_CACHE:
        _BUILD_CACHE[name] = builder()
    return _BUILD_CACHE[name]


# --------------------------------------------------------------------------
# Runners
# --------------------------------------------------------------------------
def _ensure_ntff_hook():
    """The agent image lacks antenv.axon_hooks; shim it so
    run_bass_kernel_spmd(trace=True) can capture NTFF profiles."""
    import types

    try:
        from antenv.axon_hooks import get_axon_ntff_profile_hook  # noqa: F401
        return
    except ImportError:
        pass
    try:
        import antenv
        from trn_agent_boot.trn_boot import _ntff_profile_via_ctypes

        hook = _ntff_profile_via_ctypes("/opt/axon/libaxon_pjrt.so")
        mod = types.ModuleType("antenv.axon_hooks")
        mod.get_axon_ntff_profile_hook = lambda: hook
        mod.set_axon_ntff_profile_hook = lambda h: None
        sys.modules["antenv.axon_hooks"] = mod
        antenv.axon_hooks = mod
    except Exception:
        pass


def _run(nc, in_maps, label):
    mode = os.environ.get("TRN_KERNEL_MODE", "hw")
    if mode == "sim":
        from concourse.bass_interp import CoreSim

        outs = []
        for m in in_maps:
            sim = CoreSim(nc, trace=False)
            for k, v in m.items():
                sim.tensor(k)[:] = v
            sim.simulate()
            out = {}
            for alloc in nc.m.functions[0].allocations:
                if isinstance(alloc, mybir.MemoryLocationSet) and alloc.kind == "ExternalOutput":
                    n = alloc.memorylocations[0].name
                    out[n] = np.array(sim.tensor(n))
            outs.append(out)
        LAST_EXEC_NS[label] = None
        return outs

    _ensure_ntff_hook()
    from concourse.bass_utils import run_bass_kernel_spmd

    trace = os.environ.get("TRN_KERNEL_TRACE", "1") == "1"
    t0 = time.time()
    try:
        res = run_bass_kernel_spmd(
            nc, in_maps, core_ids=list(range(N_CORES)), trace=trace
        )
    except Exception:
        if not trace:
            raise
        res = run_bass_kernel_spmd(
            nc, in_maps, core_ids=list(range(N_CORES)), trace=False
        )
    LAST_EXEC_NS[label] = res.exec_time_ns
    LAST_PROFILE[label] = res.profile_json
    LAST_EXEC_NS[label + "_wall_s"] = time.time() - t0
    return res.results


# --------------------------------------------------------------------------
# Public entry point
# --------------------------------------------------------------------------
def kernel(input_seq, last_hidden, encoder_outputs, emb, w_ih, w_hh,
           b_ih, b_hh, concat_w, concat_b, out_w_pinyin, out_b_pinyin,
           out_w_char, out_b_char):
    input_seq = np.asarray(input_seq)
    last_hidden = np.asarray(last_hidden, dtype=F32)
    E = np.asarray(encoder_outputs, dtype=F32)          # [L, B, H]
    emb = np.asarray(emb, dtype=F32)
    w_ih = np.asarray(w_ih, dtype=F32)
    w_hh = np.asarray(w_hh, dtype=F32)
    b_ih = np.asarray(b_ih, dtype=F32)
    b_hh = np.asarray(b_hh, dtype=F32)
    concat_w = np.asarray(concat_w, dtype=F32)
    concat_b = np.asarray(concat_b, dtype=F32)

    x = emb[input_seq.astype(np.int64)]                 # [B, H] gather
    h0 = last_hidden[0]                                 # [B, H]

    wrz = np.ascontiguousarray(
        np.concatenate([w_ih[: 2 * H].T, w_hh[: 2 * H].T], axis=0)
    ).astype(F16)                                       # [2H, 2H]
    wni = np.ascontiguousarray(w_ih[2 * H:].T).astype(F16)   # [H, H]
    wnh = np.ascontiguousarray(w_hh[2 * H:].T).astype(F16)   # [H, H]
    brz = np.ascontiguousarray(
        np.broadcast_to((b_ih[: 2 * H] + b_hh[: 2 * H])[None, :], (BL, 2 * H))
    ).astype(F32)
    bin_b = np.ascontiguousarray(
        np.broadcast_to(b_ih[2 * H:][None, :], (BL, H))
    ).astype(F32)
    bhn_b = np.ascontiguousarray(
        np.broadcast_to(b_hh[2 * H:][None, :], (BL, H))
    ).astype(F32)
    cwt = np.ascontiguousarray(concat_w.T).astype(F16)  # [2H, H]
    cb_b = np.ascontiguousarray(
        np.broadcast_to(concat_b[None, :], (BL, H))
    ).astype(F32)
    id16 = np.eye(128, dtype=F16)

    in_maps1 = []
    for c in range(N_CORES):
        bs = c * BL
        xc = x[bs:bs + BL]
        hc = h0[bs:bs + BL]
        xht = np.ascontiguousarray(
            np.concatenate([xc.T, hc.T], axis=0)
        ).astype(F16)                                   # [2H, BL]
        Ec = E[:, bs:bs + BL, :]                        # [L, BL, H]
        en_c = np.ascontiguousarray(Ec).astype(F16).reshape(8, 128, BL, H)
        et_c = np.ascontiguousarray(Ec.transpose(2, 1, 0)).astype(F16)
        et_c = et_c.reshape(8, 128, BL, L)
        in_maps1.append({
            "xht": xht, "h0": np.ascontiguousarray(hc), "wrz": wrz,
            "wni": wni, "wnh": wnh, "brz": brz, "bin": bin_b, "bhn": bhn_b,
            "et": et_c, "en": en_c, "cwt": cwt, "cb": cb_b, "ident": id16,
        })

    nc1 = _get("k1", _build_k1)
    res1 = _run(nc1, in_maps1, "k1")

    h_new = np.concatenate([r["hid_o"] for r in res1], axis=0)   # [B, H]
    attn = np.concatenate([r["attn_o"] for r in res1], axis=0)   # [B, L]
    co = np.concatenate([r["co_o"] for r in res1], axis=0)       # [B, H]

    # ---- kernel 2 ----
    cot = np.ascontiguousarray(co.T).astype(F16)        # [H, B]
    wp_all = np.asarray(out_w_pinyin, dtype=F32)
    wc_all = np.asarray(out_w_char, dtype=F32)
    bp_all = np.asarray(out_b_pinyin, dtype=F32)
    bc_all = np.asarray(out_b_char, dtype=F32)

    in_maps2 = []
    for c in range(N_CORES):
        vs = c * VL
        wpt = np.zeros((H, VLP), dtype=F16)
        wpt[:, :VL] = wp_all[vs:vs + VL].T.astype(F16)
        wct = np.zeros((H, VLP), dtype=F16)
        wct[:, :VL] = wc_all[vs:vs + VL].T.astype(F16)
        bp = np.zeros((B, VLP), dtype=F32)
        bp[:, :VL] = np.broadcast_to(bp_all[vs:vs + VL][None, :], (B, VL))
        bc = np.zeros((B, VLP), dtype=F32)
        bc[:, :VL] = np.broadcast_to(bc_all[vs:vs + VL][None, :], (B, VL))
        in_maps2.append({"cot": cot, "wpt": wpt, "wct": wct, "bp": bp, "bc": bc})

    nc2 = _get("k2", _build_k2)
    res2 = _run(nc2, in_maps2, "k2")

    out_pinyin = np.concatenate([r["outp"][:, :VL] for r in res2], axis=1)
    out_char = np.concatenate([r["outc"][:, :VL] for r in res2], axis=1)

    return (
        out_pinyin.astype(F32),
        out_char.astype(F32),
        h_new[None].astype(F32),
        attn[:, None, :].astype(F32),
    )
